# revision 30
# baseline (speedup 1.0000x reference)
"""Trainium2 Bass kernel for nn_DecodePredictions (YOLO-style decode, B=16).

Strategy: pure data-parallel over batch (2 images per core x 8 cores).

The reference output [B, N*C, 6] is 80x redundant: lanes 0:4 (the box) are
broadcast over the 80 classes and lane 4 is the constant class id.  The
device computes only the unique values -- boxes (exp + mul/add chain) and
the 80 per-class scores (sigmoid(obj)*sigmoid(cls)) -- and the host
replicates them into the final layout while unsharding.  That drops
per-core HBM traffic from ~36 MB (fp32 broadcast output) to ~3 MB.

Per core (P=128 partitions, KPP=132 anchors/partition, 2 images):
  in : pax  [P,KPP,8] fp16  box logits (px, py, pw+ln(s), ph+ln(s)) +
       per-anchor (s, s, gx*s, gy*s) constants, one packed tensor; the
       ln(s) fold makes Exp produce wh*s directly (one less DVE op on
       the critical tail)
       predsB [P,81*KPP] fp8e3  score logits in channel-major tile blocks
       [81, KT] so the DVE multiply sees stride-1 inner dims on all
       operands (2x perf mode; the obj broadcast rides the outer dim).
  sco: per tile: sig = Sigmoid(preds) bf16 (ACT), one DVE mul with the
       broadcast obj row -> bf16 scores out.  (A cast-to-fp8 out-DMA was
       tried and reverted: the SDMA cast path is SBUF-read-bound, so it
       moves no faster than writing bf16 directly.)
  box: epilogue after the sigmoids: wh=Exp(pwh)*s, bb[:,0]=pxy*s+grid*s,
       bb[:,1]=bb[:,0]+wh, planar fp16 out.  (Exp-first was tried and
       reverted: the serial exp-table -> exp -> sigmoid-table prologue
       delays the first sigmoid more than the epilogue's table switch
       costs, because the switch overlaps the last multiplies/stores.)
  out: bb [P,2,KPP,2] fp16 (135KB) + scores [P,80*KPP] bf16 (2.7MB).

Tiles are [20,44,36,32] anchors/partition, sized so each tile's DMA lands
just before the ACT engine finishes the previous sigmoid (no gaps).  The
last tile's multiply + store are split in half to shorten the tail.  All
DMAs ride the single HWDGE (sync) ring: leaving the SWDGE path idle keeps
its descriptor rings off the SBUF AXI ports shared with SDMA engines 7/15,
whose straggling descriptors otherwise stretch the tail by ~2us, and HWDGE
completion latency (~0.7us) is ~2x lower than SWDGE's.

Host-side: concat/pad the 3 levels to 8448 anchors, pack the per-tile
channel-major fp8 blocks, and assemble the full [B, N*C, 6] fp32 output
from the compact device outputs.
"""

import ml_dtypes
import numpy as np

N_CORES = 8
B = 16
B_PER_CORE = B // N_CORES  # 2
C = 80
F = 85
CH = 81                    # obj + 80 cls
N_REAL = 8400              # 80*80 + 40*40 + 20*20
N_PAD = 8448               # = 66 * 128
P = 128
KPP = B_PER_CORE * N_PAD // P  # 132 anchors per partition
KTS = (20, 44, 36, 32)     # score-tile sizes (anchors/partition)
OFFS = (0, 20, 64, 100)

_CACHE: dict = {}


def _build_nc():
    import concourse.bacc as bacc
    import concourse.tile as tile
    from concourse import mybir
    from contextlib import ExitStack

    nc = bacc.Bacc("TRN2", target_bir_lowering=False, debug=False)
    pax = nc.dram_tensor("pax", [P, KPP, 8], mybir.dt.float16, kind="ExternalInput")
    predsB = nc.dram_tensor("predsB", [P, CH * KPP], mybir.dt.float8e3, kind="ExternalInput")
    bb = nc.dram_tensor("bb", [P, 2, KPP, 2], mybir.dt.float16, kind="ExternalOutput")
    scores = nc.dram_tensor("scores", [P, C * KPP], mybir.dt.bfloat16, kind="ExternalOutput")

    fp16 = mybir.dt.float16
    bf16 = mybir.dt.bfloat16
    fp8 = mybir.dt.float8e3
    AF = mybir.ActivationFunctionType

    with tile.TileContext(nc) as tc, ExitStack() as ctx:
        cpool = ctx.enter_context(tc.tile_pool(name="const", bufs=1))
        spool = ctx.enter_context(tc.tile_pool(name="sig", bufs=2))
        opool = ctx.enter_context(tc.tile_pool(name="sc", bufs=2))

        # Preds tiles first on the HWDGE queue: the first sigmoid gates
        # everything downstream, so its (small) tile leads.
        pt = []
        for t, kt in enumerate(KTS):
            ptile = cpool.tile([P, CH, kt], fp8, tag=f"pt{t}", name=f"pt{t}")
            nc.sync.dma_start(
                out=ptile[:], in_=predsB[:, CH * OFFS[t] : CH * (OFFS[t] + kt)]
            )
            pt.append(ptile)
        pax_t = cpool.tile([P, KPP, 8], fp16, tag="pax")
        nc.sync.dma_start(out=pax_t[:], in_=pax[:])

        for t, kt in enumerate(KTS[:-1]):
            sig = spool.tile([P, CH, kt], bf16, tag=f"sig{t % 2}", name=f"sig{t}")
            nc.scalar.activation(sig[:], pt[t][:], AF.Sigmoid)
            sc = opool.tile([P, C, kt], bf16, tag=f"sc{t % 2}", name=f"sc{t}")
            nc.vector.tensor_mul(
                sc[:],
                sig[:, 1:CH, :],
                sig[:, 0:1, :].broadcast_to([P, C, kt]),
            )
            # Early stores ride SWDGE so the HWDGE ring is clear for the
            # tail; SWDGE stragglers finish well before the tail lands.
            eng = nc.gpsimd if t < 2 else nc.sync
            eng.dma_start(
                out=scores[:, C * OFFS[t] : C * (OFFS[t] + kt)], in_=sc[:]
            )

        # Last tile: multiply + store in two halves on the fast-receipt
        # HWDGE ring so the tail is two short dependency chains.
        t3, kt3 = len(KTS) - 1, KTS[-1]
        kh = kt3 // 2
        sig3 = spool.tile([P, CH, kt3], bf16, tag="sig1", name="sig3")
        nc.scalar.activation(sig3[:], pt[t3][:], AF.Sigmoid)
        for h in range(2):
            hs = slice(h * kh, (h + 1) * kh)
            sch = opool.tile([P, C, kh], bf16, tag=f"sch{h}", name=f"sch{h}")
            nc.vector.tensor_mul(
                sch[:],
                sig3[:, 1:CH, hs],
                sig3[:, 0:1, hs].broadcast_to([P, C, kh]),
            )
            o0 = C * (OFFS[t3] + h * kh)
            nc.sync.dma_start(out=scores[:, o0 : o0 + C * kh], in_=sch[:])

        # Box epilogue: one ACT table switch to Exp after the sigmoids; the
        # chain overlaps the last score multiplies and stores.  Plane 0
        # (x1,y1) needs no Exp and stores mid-stream; only the small
        # (x2,y2) plane trails the Exp.
        bb_t = cpool.tile([P, 2, KPP, 2], fp16, tag="bb")
        nc.vector.tensor_mul(bb_t[:, 0, :, :], pax_t[:, :, 0:2], pax_t[:, :, 4:6])
        nc.vector.tensor_add(bb_t[:, 0, :, :], bb_t[:, 0, :, :], pax_t[:, :, 6:8])
        nc.sync.dma_start(out=bb[:, 0, :, :], in_=bb_t[:, 0, :, :])
        # Host stores pw+ln(s)-2 (centered to keep fp16 ulp small); the
        # fp32-internal bias restores the 2.
        bias2 = cpool.tile([P, 1], mybir.dt.float32, tag="bias2")
        nc.gpsimd.memset(bias2[:], 2.0)
        wh_t = cpool.tile([P, KPP, 2], fp16, tag="wh")
        nc.scalar.activation(wh_t[:], pax_t[:, :, 2:4], AF.Exp, bias=bias2[:])
        nc.vector.tensor_add(bb_t[:, 1, :, :], bb_t[:, 0, :, :], wh_t[:])
        nc.sync.dma_start(out=bb[:, 1, :, :], in_=bb_t[:, 1, :, :])

    nc.compile()
    return nc


def _host_consts():
    # Per-anchor (stride, stride, gx*stride, gy*stride), padded to N_PAD.
    s = np.ones(N_PAD, np.float32)
    bx = np.zeros(N_PAD, np.float32)
    by = np.zeros(N_PAD, np.float32)
    off = 0
    for g, st in ((80, 8.0), (40, 16.0), (20, 32.0)):
        n = g * g
        i = np.arange(n)
        s[off : off + n] = st
        bx[off : off + n] = (i % g) * st
        by[off : off + n] = (i // g) * st
        off += n
    auxp = np.stack([s, s, bx, by], axis=-1).astype(np.float16)
    auxp = np.concatenate([auxp] * B_PER_CORE, 0).reshape(P, KPP, 4)
    return np.ascontiguousarray(auxp)  # packed into pax[..., 4:8] per call


def _host_in_maps(pred0, pred1, pred2):
    auxp = _CACHE["consts"]
    pred0 = np.asarray(pred0, np.float32).reshape(B, -1, F)
    pred1 = np.asarray(pred1, np.float32).reshape(B, -1, F)
    pred2 = np.asarray(pred2, np.float32).reshape(B, -1, F)
    in_maps = []
    for core in range(N_CORES):
        flat = np.zeros((B_PER_CORE * N_PAD, F), np.float32)
        for j in range(B_PER_CORE):
            b = core * B_PER_CORE + j
            flat[j * N_PAD : j * N_PAD + N_REAL] = np.concatenate(
                [pred0[b], pred1[b], pred2[b]], axis=0
            )
        # Channel-major per tile: block t is [CH, KT_t] per partition.
        lg = (
            flat[:, 4:F]
            .astype(ml_dtypes.float8_e3m4)
            .reshape(P, KPP, CH)
        )
        blocks = [
            np.ascontiguousarray(lg[:, OFFS[t] : OFFS[t] + kt, :].transpose(0, 2, 1))
            for t, kt in enumerate(KTS)
        ]
        predsB = np.concatenate([b.reshape(P, -1) for b in blocks], axis=1)
        pax = np.empty((P, KPP, 8), np.float16)
        pax[:, :, 0:4] = flat[:, 0:4].astype(np.float16).reshape(P, KPP, 4)
        pax[:, :, 4:8] = auxp
        # Fold ln(s)-2 into the wh logits (fp32 math, then one fp16 round):
        # Exp(x + 2) on device then yields wh*s directly.
        pax[:, :, 2:4] = (
            flat[:, 2:4].reshape(P, KPP, 2)
            + np.log(auxp[:, :, 0:2].astype(np.float32))
            - 2.0
        ).astype(np.float16)
        in_maps.append(
            {
                "pax": pax,
                "predsB": np.ascontiguousarray(predsB),
            }
        )
    return in_maps


def kernel(images, pred0, pred1, pred2):
    from concourse.bass_utils import run_bass_kernel_spmd

    if "nc" not in _CACHE:
        _CACHE["consts"] = _host_consts()
        _CACHE["nc"] = _build_nc()
    nc = _CACHE["nc"]

    in_maps = _host_in_maps(pred0, pred1, pred2)
    res = run_bass_kernel_spmd(nc, in_maps, list(range(N_CORES)))

    full = np.empty((B, N_REAL, C, 6), np.float32)
    full[:, :, :, 4] = np.arange(C, dtype=np.float32)
    for core, r in enumerate(res.results):
        b0 = core * B_PER_CORE
        boxes = (
            np.asarray(r["bb"])
            .astype(np.float32)
            .transpose(0, 2, 1, 3)
            .reshape(B_PER_CORE, N_PAD, 4)[:, :N_REAL]
        )
        sc_flat = np.asarray(r["scores"])  # [P, C*KPP] fp8e3 in tile blocks
        parts = []
        for t, kt in enumerate(KTS):
            blk = sc_flat[:, C * OFFS[t] : C * (OFFS[t] + kt)].reshape(P, C, kt)
            parts.append(blk.transpose(0, 2, 1))  # [P, kt, C]
        sc = (
            np.concatenate(parts, axis=1)
            .astype(np.float32)
            .reshape(B_PER_CORE, N_PAD, C)[:, :N_REAL]
        )
        full[b0 : b0 + B_PER_CORE, :, :, 0:4] = boxes[:, :, None, :]
        full[b0 : b0 + B_PER_CORE, :, :, 5] = sc
    return full.reshape(B, N_REAL * C, 6)
